# revision 4
# baseline (speedup 1.0000x reference)
"""EuclideanCodebook (VQ) kernel for 8 Trainium2 NeuronCores.

Computes, for x [B=8, N=2048, D=256] and codebook embed [1, C=8192, D=256]:
    quantize [B, N, D] = embed[argmin_c ||x - e_c||^2]
    embed_ind [B, N]   = argmin indices (int32)

Sharding: data-parallel over B (one batch row of 2048 tokens per core),
codebook replicated.

Numerical contract (matches jax fp32 reference bit-for-bit on argmin):
the reference ranks codes by d2 = fl(fl(x2 - 2*xe) + e2).  Because x2 ~ 256
dominates, d2 is quantized to ~2^-16; near-ties are broken by lowest index.
We reproduce the identical two-step rounding chain with s = -d2:
    u = fl(2*xe - x2)      (ACT: Copy(2*psum + (-x2)); *2 exact => same fl)
    s = fl(u - e2)         (DVE tensor_tensor_reduce, fused row-max -> gmax)
    idx = first position with s == gmax   (DVE max_index: lowest-index ties)
xe itself only needs fp32-level accuracy (verified: any fp32-order accum and
even bf16 hi/lo 3-term split give 0 argmin flips vs the reference).
"""

import sys

if "/opt/trn_rl_repo" not in sys.path:
    sys.path.insert(0, "/opt/trn_rl_repo")

from contextlib import ExitStack

import numpy as np

B, N, D = 8, 2048, 256
C = 8192
MLOC = N          # tokens per core
P = 128           # partitions
NT = MLOC // P    # 16 m-tiles per core
CTW = 512         # c-tile width (one PSUM bank, fp32)
NCT = C // CTW    # 16 c-tiles
KC = D // P       # 2 contraction chunks

# MM_DTYPE: "f32" (native fp32 matmul, 4 cyc/row), "split3" (bf16 hi/lo
# 3-term split, 3 cyc/row), "f32r" (float32r, 1 cyc/row, precision TBD)
MM_MODE = "split3"

_BUILD_CACHE = {}


def _build_program(mm_mode=MM_MODE):
    import concourse.bacc as bacc
    import concourse.bass as bass
    import concourse.tile as tile
    from concourse import mybir

    fp32 = mybir.dt.float32
    bf16 = mybir.dt.bfloat16

    nc = bacc.Bacc(None, target_bir_lowering=False, debug=False,
                   enable_asserts=False)

    # DRAM I/O (per core; SPMD same program).
    if mm_mode == "split3":
        xT_h = nc.dram_tensor("xTh", [D, MLOC], bf16, kind="ExternalInput")
        xT_l = nc.dram_tensor("xTl", [D, MLOC], bf16, kind="ExternalInput")
        eT_h = nc.dram_tensor("eTh", [D, C], bf16, kind="ExternalInput")
        eT_l = nc.dram_tensor("eTl", [D, C], bf16, kind="ExternalInput")
    else:
        mmdt = fp32 if mm_mode == "f32" else mybir.dt.float32r
        xT_d = nc.dram_tensor("xT", [D, MLOC], mmdt, kind="ExternalInput")
        eT_d = nc.dram_tensor("eT", [D, C], mmdt, kind="ExternalInput")
    nx2_d = nc.dram_tensor("nx2", [MLOC, 1], fp32, kind="ExternalInput")
    e2_d = nc.dram_tensor("e2", [C], fp32, kind="ExternalInput")
    emb_d = nc.dram_tensor("emb", [C, D], fp32, kind="ExternalInput")
    qout_d = nc.dram_tensor("qout", [MLOC, D], fp32, kind="ExternalOutput")
    iout_d = nc.dram_tensor("iout", [MLOC, 1], mybir.dt.int32,
                            kind="ExternalOutput")

    with ExitStack() as ctx:
        tc = ctx.enter_context(tile.TileContext(nc))
        consts = ctx.enter_context(tc.tile_pool(name="consts", bufs=1))
        mwork = ctx.enter_context(tc.tile_pool(name="mwork", bufs=1))
        lhs = ctx.enter_context(tc.tile_pool(name="lhs", bufs=3))
        small = ctx.enter_context(tc.tile_pool(name="small", bufs=4))
        psum = ctx.enter_context(
            tc.tile_pool(name="psum", bufs=8, space="PSUM"))

        # Replicated constants in SBUF.
        if mm_mode == "split3":
            eTh_sb = consts.tile([P, KC, C], bf16)
            eTl_sb = consts.tile([P, KC, C], bf16)
            for k in range(KC):
                nc.sync.dma_start(out=eTh_sb[:, k, :],
                                  in_=eT_h[k * P:(k + 1) * P, :])
                nc.sync.dma_start(out=eTl_sb[:, k, :],
                                  in_=eT_l[k * P:(k + 1) * P, :])
        else:
            eT_sb = consts.tile([P, KC, C], eT_d.dtype)
            for k in range(KC):
                nc.sync.dma_start(out=eT_sb[:, k, :],
                                  in_=eT_d[k * P:(k + 1) * P, :])
        e2b = consts.tile([P, C], fp32)
        e2_ap = e2_d.ap()
        e2_bcast = bass.AP(tensor=e2_ap.tensor, offset=e2_ap.offset,
                           ap=[[0, P]] + list(e2_ap.ap))
        nc.sync.dma_start(out=e2b, in_=e2_bcast)

        for t in range(NT):
            msl = slice(t * P, (t + 1) * P)
            if mm_mode == "split3":
                xh_t = lhs.tile([P, KC, P], bf16, tag="xh")
                xl_t = lhs.tile([P, KC, P], bf16, tag="xl")
                for k in range(KC):
                    nc.sync.dma_start(out=xh_t[:, k, :],
                                      in_=xT_h[k * P:(k + 1) * P, msl])
                    nc.sync.dma_start(out=xl_t[:, k, :],
                                      in_=xT_l[k * P:(k + 1) * P, msl])
            else:
                xt_t = lhs.tile([P, KC, P], xT_d.dtype, tag="xt")
                for k in range(KC):
                    nc.sync.dma_start(out=xt_t[:, k, :],
                                      in_=xT_d[k * P:(k + 1) * P, msl])
            nx2c = small.tile([P, 1], fp32, tag="nx2c")
            nc.sync.dma_start(out=nx2c, in_=nx2_d[msl, :])

            u = mwork.tile([P, C], fp32, tag="u")
            for j in range(NCT):
                csl = slice(j * CTW, (j + 1) * CTW)
                ps = psum.tile([P, CTW], fp32, tag="ps")
                if mm_mode == "split3":
                    # xe ~= xh@eh + xh@el + xl@eh (xl@el term ~2^-18, dropped)
                    nc.tensor.matmul(ps, xh_t[:, 0, :], eTh_sb[:, 0, csl],
                                     start=True, stop=False)
                    nc.tensor.matmul(ps, xh_t[:, 1, :], eTh_sb[:, 1, csl],
                                     start=False, stop=False)
                    nc.tensor.matmul(ps, xh_t[:, 0, :], eTl_sb[:, 0, csl],
                                     start=False, stop=False)
                    nc.tensor.matmul(ps, xh_t[:, 1, :], eTl_sb[:, 1, csl],
                                     start=False, stop=False)
                    nc.tensor.matmul(ps, xl_t[:, 0, :], eTh_sb[:, 0, csl],
                                     start=False, stop=False)
                    nc.tensor.matmul(ps, xl_t[:, 1, :], eTh_sb[:, 1, csl],
                                     start=False, stop=True)
                else:
                    nc.tensor.matmul(ps, xt_t[:, 0, :], eT_sb[:, 0, csl],
                                     start=True, stop=False)
                    nc.tensor.matmul(ps, xt_t[:, 1, :], eT_sb[:, 1, csl],
                                     start=False, stop=True)
                # u = fl(2*xe - x2): *2 exact, one fp32 rounding -> identical
                # to the reference's fl(x2 - 2*xe) negated.  Identity (not
                # Copy) because only Identity takes a per-partition bias AP.
                nc.scalar.activation(u[:, csl], ps,
                                     mybir.ActivationFunctionType.Identity,
                                     bias=nx2c, scale=2.0)

            # s = fl(u - e2) (= -d2, ref rounding chain); gmax = row max.
            # (fused tensor_tensor_reduce fails NEFF lowering on the axon
            # path, so use separate tensor_tensor + tensor_reduce.)
            s = mwork.tile([P, C], fp32, tag="s")
            gmax = small.tile([P, 1], fp32, tag="gmax")
            nc.vector.tensor_tensor(s, u, e2b, op=mybir.AluOpType.subtract)
            nc.vector.tensor_reduce(gmax, s, axis=mybir.AxisListType.X,
                                    op=mybir.AluOpType.max)

            # First (lowest) index where s == gmax: ref argmin tie semantics.
            needles = small.tile([P, 8], fp32, tag="needles")
            nc.vector.memset(needles, 3.0e38)
            nc.vector.tensor_copy(needles[:, 0:1], gmax)
            idx8 = small.tile([P, 8], mybir.dt.uint32, tag="idx8")
            nc.vector.max_index(idx8, needles, s)
            idxi = small.tile([P, 1], mybir.dt.int32, tag="idxi")
            nc.vector.tensor_copy(idxi, idx8[:, 0:1])
            nc.sync.dma_start(out=iout_d[msl, :], in_=idxi)

            # Gather the chosen codebook rows.
            qrow = small.tile([P, D], fp32, tag="qrow")
            nc.gpsimd.indirect_dma_start(
                out=qrow, out_offset=None, in_=emb_d.ap(),
                in_offset=bass.IndirectOffsetOnAxis(ap=idx8[:, 0:1], axis=0))
            nc.sync.dma_start(out=qout_d[msl, :], in_=qrow)

    nc.compile()
    return nc


def _to_bf16_pair(a):
    """Split fp32 array into bf16 hi + bf16 lo with hi = rne(a)."""
    import ml_dtypes
    hi = a.astype(ml_dtypes.bfloat16)
    lo = (a - hi.astype(np.float32)).astype(ml_dtypes.bfloat16)
    return hi, lo


def _prep_inputs(x, embed, mm_mode=MM_MODE):
    x = np.ascontiguousarray(np.asarray(x, dtype=np.float32))
    e = np.ascontiguousarray(np.asarray(embed, dtype=np.float32)[0])  # [C, D]
    eT = np.ascontiguousarray(e.T)                                    # [D, C]
    e2 = np.sum(e * e, axis=-1, dtype=np.float32)                     # [C]
    common = {"e2": e2, "emb": e}
    if mm_mode == "split3":
        eTh, eTl = _to_bf16_pair(eT)
        common["eTh"] = np.ascontiguousarray(eTh)
        common["eTl"] = np.ascontiguousarray(eTl)
    else:
        common["eT"] = eT
    in_maps = []
    for b in range(B):
        xb = x[b]                                                     # [N, D]
        xT = np.ascontiguousarray(xb.T)                               # [D, N]
        nx2 = (-np.sum(xb * xb, axis=-1, dtype=np.float32)).reshape(MLOC, 1)
        m = dict(common)
        m["nx2"] = nx2
        if mm_mode == "split3":
            xTh, xTl = _to_bf16_pair(xT)
            m["xTh"] = np.ascontiguousarray(xTh)
            m["xTl"] = np.ascontiguousarray(xTl)
        else:
            m["xT"] = xT
        in_maps.append(m)
    return in_maps


def kernel(x, embed):
    from concourse import bass_utils

    mm_mode = MM_MODE
    if mm_mode not in _BUILD_CACHE:
        _BUILD_CACHE[mm_mode] = _build_program(mm_mode)
    nc = _BUILD_CACHE[mm_mode]

    in_maps = _prep_inputs(x, embed, mm_mode)
    res = bass_utils.run_bass_kernel_spmd(nc, in_maps,
                                          core_ids=list(range(B)))
    quantize = np.stack([res.results[b]["qout"] for b in range(B)], axis=0)
    embed_ind = np.stack(
        [res.results[b]["iout"][:, 0] for b in range(B)], axis=0)
    return quantize.astype(np.float32), embed_ind.astype(np.int32)
